# revision 9
# baseline (speedup 1.0000x reference)
"""Trainium2 Bass kernel for nn_Disentangler (gnn_message_passing).

Math (per timestamp t, fully data-parallel across 8 cores):
  xn   = LN(x[t, :8192], ln1_g, ln1_b)
  tee  = scatter_add(xn by indices[t]) into 32768 slots
  h    = gelu(tee @ w1 + b1) @ w2 + b2
  comp = LNf(chunk_sum(h))                       # 16 chunks of 2048 slots
  dec  = (gelu(LNd(comp_rows) @ dw1 + db1) @ dw2 + db2)   # only 16 distinct rows!
  out[t, i] = dec[indices[t, i] >> 11]; out[t, 8192:] = 0

Key transforms vs the reference:
  * LN folds into the first matmul: a_i = r_i * ((x_i - m_i) @ (g*w1)) [+ b@w1]
  * scatter-add runs at the 128-dim `a` level on *unique* slots only, via
    multiplicity-ordered dma_gather rounds from a DRAM spill of `a`.
  * chunk-sum (incl. empty-slot correction) is a matmul against a host-built
    per-chunk membership matrix M^T.
  * decode MLP computed on 16 rows; the final gather is a one-hot matmul with
    a split-bf16 (hi+lo) trick for full fp32 precision.
"""

import math

import numpy as np
import ml_dtypes

import concourse.bacc as bacc
import concourse.mybir as mybir
import concourse.tile as tile
import concourse.bass_utils as bass_utils
from concourse.masks import make_identity

# problem constants
T, NUM_TOKENS, D = 8, 12288, 256
N_NODE, NUM_NODES = 8192, 32768
L, C, H = 16, 64, 128          # COMP_LEN, COMP_DIM, 2*COMP_DIM
CHUNK = NUM_NODES // L         # 2048
P = 128
B = 8                          # token blocks per bigtile (1024 tokens / bigtile)
NBT = N_NODE // (P * B)        # 8 bigtiles
ZROW = N_NODE                  # zero row in the a-spill table
SINGLE_PACKET = False
EPS = 1e-5
F32 = mybir.dt.float32
BF16 = mybir.dt.bfloat16
I16 = mybir.dt.int16
AF = mybir.ActivationFunctionType
ALU = mybir.AluOpType


def _pack16(v):
    """int index list (len % 16 == 0) -> [128, n/16] int16 (wrap 16, replicate)."""
    a = np.asarray(v, np.int16).reshape(-1, 16).T
    return np.ascontiguousarray(np.tile(a, (8, 1)))


def host_prep(x, indices, ln1_g, ln1_b, w1, b1, w2, b2,
              lnf_g, lnf_b, lnd_g, lnd_b, dw1, db1, dw2, db2):
    """Build per-core in_maps + global config."""
    f = np.float32
    x = np.asarray(x, f)
    ln1_g, ln1_b = np.asarray(ln1_g, f), np.asarray(ln1_b, f)
    w1, b1 = np.asarray(w1, f), np.asarray(b1, f)
    w2, b2 = np.asarray(w2, f), np.asarray(b2, f)
    lnf_g, lnf_b = np.asarray(lnf_g, f), np.asarray(lnf_b, f)
    lnd_g, lnd_b = np.asarray(lnd_g, f), np.asarray(lnd_b, f)
    dw1, db1 = np.asarray(dw1, f), np.asarray(db1, f)
    dw2, db2 = np.asarray(dw2, f), np.asarray(db2, f)

    per_t = []
    for t in range(T):
        idx = np.asarray(indices[t], np.int64)
        uniq, counts = np.unique(idx, return_counts=True)
        order = np.argsort(-counts, kind="stable")
        sp = np.argsort(idx, kind="stable")
        starts = np.zeros(uniq.size + 1, np.int64)
        starts[1:] = np.cumsum(counts)
        per_t.append(dict(idx=idx, uniq=uniq, counts=counts, order=order,
                          sp=sp, starts=starts, U=uniq.size,
                          K=int(counts.max())))

    U_max = max(d["U"] for d in per_t)
    U_pad = P * math.ceil((U_max + L) / P)
    NB = U_pad // P
    K_g = max(d["K"] for d in per_t)
    # per collision round k >= 1: global padded sizes
    mks = []
    for k in range(1, K_g):
        mk = max(int((d["counts"] > k).sum()) for d in per_t)
        mks.append(P * math.ceil(mk / P))

    cfg = dict(
        U_pad=U_pad, NB=NB, K_g=K_g, mks=mks,
        has_bw1=bool(np.any(ln1_b != 0)),
        has_b1=bool(np.any(b1 != 0)),
        lnf_triv=bool(np.all(lnf_g == 1) and np.all(lnf_b == 0)),
        lnd_triv=bool(np.all(lnd_g == 1) and np.all(lnd_b == 0)),
    )

    W1g = (ln1_g[:, None] * w1).reshape(2, P, H)
    shared = {
        "w1g": np.ascontiguousarray(W1g),
        "w2": np.ascontiguousarray(w2),                       # [128, 64]
        "b2r": np.ascontiguousarray((CHUNK * b2)[None, :]),   # [1, 64]
        "dw1": np.ascontiguousarray(dw1),                     # [64, 128]
        "db1r": np.ascontiguousarray(db1[None, :]),           # [1, 128]
        "dw2": np.ascontiguousarray(dw2),                     # [128, 256]
        "db2r": np.ascontiguousarray(db2[None, :]),           # [1, 256]
    }
    if not cfg["lnf_triv"]:
        shared["lnfg"] = np.ascontiguousarray(lnf_g.reshape(L, C).T)
        shared["lnfb"] = np.ascontiguousarray(lnf_b.reshape(L, C).T)
    if not cfg["lnd_triv"]:
        shared["lndg"] = np.ascontiguousarray(np.tile(lnd_g, (L, 1)))
        shared["lndb"] = np.ascontiguousarray(np.tile(lnd_b, (L, 1)))
    if cfg["has_bw1"]:
        shared["bw1rep"] = np.ascontiguousarray(
            np.tile((ln1_b @ w1)[None, :], (P, 1)))
    if cfg["has_b1"]:
        shared["b1rep"] = np.ascontiguousarray(np.tile(b1[None, :], (P, 1)))

    in_maps = []
    for t in range(T):
        d = per_t[t]
        idx, uniq, counts, order = d["idx"], d["uniq"], d["counts"], d["order"]
        sp, starts, U = d["sp"], d["starts"], d["U"]

        g0 = np.full(U_pad, ZROW, np.int64)
        g0[:U] = sp[starts[order]]
        gks = []
        for k in range(1, K_g):
            gk = np.full(mks[k - 1], ZROW, np.int64)
            sel = counts[order] > k          # prefix (desc multiplicity sort)
            nsel = int(sel.sum())
            if nsel:
                gk[:nsel] = sp[starts[order[sel]] + k]
            gks.append(gk)

        mt = np.zeros((U_pad, L), np.float32)
        lu = (uniq >> 11).astype(np.int64)
        mt[np.arange(U), lu[order]] = 1.0
        cnt_chunk = np.bincount(lu, minlength=L).astype(np.float32)
        mt[U + np.arange(L), np.arange(L)] = CHUNK - cnt_chunk
        # device layout [128, NB, L]: row j = blk*128 + p
        mt_dev = np.ascontiguousarray(
            mt.reshape(NB, P, L).transpose(1, 0, 2))

        l_arr = (idx >> 11).astype(np.int64)
        lv = l_arr.reshape(NBT, P, B)                  # token = bt*1024 + p*8 + b
        oh = np.zeros((4 * L, NBT, B, P), np.float32)
        bt_i, p_i, b_i = np.indices((NBT, P, B))
        oh[lv, bt_i, b_i, p_i] = 1.0
        oh[lv + 2 * L, bt_i, b_i, p_i] = 1.0
        oh_dev = oh.reshape(4 * L, N_NODE).astype(ml_dtypes.bfloat16)

        m = {
            "xt": np.ascontiguousarray(x[t, :N_NODE, :]),
            "mt": mt_dev,
            "oh": oh_dev,
            "g0": _pack16(g0),
        }
        for k in range(1, K_g):
            m[f"g{k}"] = _pack16(gks[k - 1])
        m.update(shared)
        in_maps.append(m)
    return cfg, in_maps


def build(cfg, loop_k=0, phase='all'):
    """Build the Bass program. loop_k>0 wraps the body in a hardware loop
    (for timing); loop_k=0 emits a single-shot kernel."""
    U_pad, NB, K_g, mks = cfg["U_pad"], cfg["NB"], cfg["K_g"], cfg["mks"]
    nc = bacc.Bacc("TRN2", num_devices=8)

    xt = nc.dram_tensor("xt", [N_NODE, D], F32, kind="ExternalInput").ap()
    w1g = nc.dram_tensor("w1g", [2, P, H], F32, kind="ExternalInput").ap()
    mt_d = nc.dram_tensor("mt", [P, NB, L], F32, kind="ExternalInput").ap()
    oh_d = nc.dram_tensor("oh", [4 * L, N_NODE], BF16, kind="ExternalInput").ap()
    g0_d = nc.dram_tensor("g0", [P, U_pad // 16], I16, kind="ExternalInput").ap()
    gk_d = [nc.dram_tensor(f"g{k}", [P, mks[k - 1] // 16], I16,
                           kind="ExternalInput").ap() for k in range(1, K_g)]
    w2_d = nc.dram_tensor("w2", [H, C], F32, kind="ExternalInput").ap()
    b2r_d = nc.dram_tensor("b2r", [1, C], F32, kind="ExternalInput").ap()
    dw1_d = nc.dram_tensor("dw1", [C, H], F32, kind="ExternalInput").ap()
    db1r_d = nc.dram_tensor("db1r", [1, H], F32, kind="ExternalInput").ap()
    dw2_d = nc.dram_tensor("dw2", [H, D], F32, kind="ExternalInput").ap()
    db2r_d = nc.dram_tensor("db2r", [1, D], F32, kind="ExternalInput").ap()
    opt = {}
    if not cfg["lnf_triv"]:
        opt["lnfg"] = nc.dram_tensor("lnfg", [C, L], F32, kind="ExternalInput").ap()
        opt["lnfb"] = nc.dram_tensor("lnfb", [C, L], F32, kind="ExternalInput").ap()
    if not cfg["lnd_triv"]:
        opt["lndg"] = nc.dram_tensor("lndg", [L, C], F32, kind="ExternalInput").ap()
        opt["lndb"] = nc.dram_tensor("lndb", [L, C], F32, kind="ExternalInput").ap()
    if cfg["has_bw1"]:
        opt["bw1rep"] = nc.dram_tensor("bw1rep", [P, H], F32, kind="ExternalInput").ap()
    if cfg["has_b1"]:
        opt["b1rep"] = nc.dram_tensor("b1rep", [P, H], F32, kind="ExternalInput").ap()

    out_d = nc.dram_tensor("out", [NUM_TOKENS, D], F32, kind="ExternalOutput").ap()
    adram = nc.dram_tensor("adram", [N_NODE + 1, H], F32, kind="Internal").ap()

    with tile.TileContext(nc) as tc:
        with (
            tc.tile_pool(name="const", bufs=1) as cpool,
            tc.tile_pool(name="x", bufs=2) as xpool,
            tc.tile_pool(name="stats", bufs=2) as spool,
            tc.tile_pool(name="xT", bufs=3) as xtpool,
            tc.tile_pool(name="a", bufs=2) as apool,
            tc.tile_pool(name="acc", bufs=1) as accpool,
            tc.tile_pool(name="stg", bufs=2) as stgpool,
            tc.tile_pool(name="dec", bufs=1) as dpool,
            tc.tile_pool(name="outp", bufs=2) as opool,
            tc.tile_pool(name="ps_tr", bufs=2, space="PSUM") as ps_tr,
            tc.tile_pool(name="ps_mm", bufs=2, space="PSUM") as ps_mm,
            tc.tile_pool(name="ps_cs", bufs=1, space="PSUM") as ps_cs,
            tc.tile_pool(name="ps_out", bufs=2, space="PSUM") as ps_out,
            tc.tile_pool(name="ps_sm", bufs=1, space="PSUM") as ps_sm,
        ):
            # ---------- constants ----------
            ident = cpool.tile([P, P], F32)
            make_identity(nc, ident[:])
            zt = cpool.tile([P, 2048], F32)
            nc.vector.memset(zt[:], 0.0)
            ones16 = cpool.tile([1, L], F32)
            nc.vector.memset(ones16[:], 1.0)
            onescol = cpool.tile([C, 1], F32)
            nc.vector.memset(onescol[:], 1.0)

            w1g_sb = cpool.tile([P, 2, H], F32)
            nc.sync.dma_start(out=w1g_sb[:], in_=w1g[:].rearrange("k p h -> p k h"))
            w2_sb = cpool.tile([H, C], F32)
            nc.sync.dma_start(out=w2_sb[:], in_=w2_d[:])
            b2r_sb = cpool.tile([1, C], F32)
            nc.sync.dma_start(out=b2r_sb[:], in_=b2r_d[:])
            dw1_sb = cpool.tile([C, H], F32)
            nc.sync.dma_start(out=dw1_sb[:], in_=dw1_d[:])
            db1r_sb = cpool.tile([1, H], F32)
            nc.sync.dma_start(out=db1r_sb[:], in_=db1r_d[:])
            dw2_sb = cpool.tile([H, D], F32)
            nc.sync.dma_start(out=dw2_sb[:], in_=dw2_d[:])
            db2r_sb = cpool.tile([1, D], F32)
            nc.sync.dma_start(out=db2r_sb[:], in_=db2r_d[:])
            mt_sb = cpool.tile([P, NB, L], F32)
            nc.sync.dma_start(out=mt_sb[:], in_=mt_d[:])
            oh_sb = cpool.tile([4 * L, N_NODE], BF16)
            nc.sync.dma_start(out=oh_sb[:], in_=oh_d[:])
            g0_sb = cpool.tile([P, U_pad // 16], I16)
            nc.sync.dma_start(out=g0_sb[:], in_=g0_d[:])
            gk_sb = []
            for k in range(1, K_g):
                gt = cpool.tile([P, mks[k - 1] // 16], I16, tag=f"gk{k}")
                nc.sync.dma_start(out=gt[:], in_=gk_d[k - 1][:])
                gk_sb.append(gt)
            osb = {}
            for name, ap in opt.items():
                t_ = cpool.tile(list(ap.shape), F32, tag=name)
                nc.sync.dma_start(out=t_[:], in_=ap[:])
                osb[name] = t_
            # zero row of the a-table
            nc.sync.dma_start(out=adram[ZROW:ZROW + 1, :], in_=zt[0:1, 0:H])

            def body(_i=None):
                # ---------- encode: LN-folded matmul + spill ----------
                for bt in range(NBT):
                    xb = xpool.tile([P, B, D], F32, tag="xb")
                    nc.sync.dma_start(
                        out=xb[:], in_=xt[bt * 1024:(bt + 1) * 1024, :])
                    st = spool.tile([P, B, 6], F32, tag="st")
                    mv = spool.tile([P, B, 2], F32, tag="mv")
                    for b in range(B):
                        nc.vector.bn_stats(st[:, b, :], xb[:, b, :])
                    for b in range(B):
                        nc.vector.bn_aggr(mv[:, b, :], st[:, b, :])
                    rc = spool.tile([P, B], F32, tag="rc")
                    nm = spool.tile([P, B], F32, tag="nm")
                    nc.vector.tensor_scalar_add(rc[:], mv[:, :, 1], EPS)
                    nc.scalar.sqrt(rc[:], rc[:])
                    nc.vector.reciprocal(rc[:], rc[:])
                    nc.vector.tensor_scalar_mul(nm[:], mv[:, :, 0], -1.0)
                    a_big = apool.tile([P, B, H], F32, tag="a")
                    for b in range(B):
                        nc.scalar.activation(
                            xb[:, b, :], xb[:, b, :], AF.Identity,
                            bias=nm[:, b:b + 1])
                        trp = ps_tr.tile([P, D], F32, space="PSUM", tag="trp")
                        nc.tensor.transpose(
                            out=trp[:, 0:P], in_=xb[:, b, 0:P], identity=ident[:])
                        nc.tensor.transpose(
                            out=trp[:, P:D], in_=xb[:, b, P:D], identity=ident[:])
                        xTs = xtpool.tile([P, D], F32, tag="xT")
                        if b % 2 == 0:
                            nc.vector.tensor_copy(out=xTs[:], in_=trp[:])
                        else:
                            nc.scalar.copy(out=xTs[:], in_=trp[:])
                        pp = ps_mm.tile([P, H], F32, space="PSUM", tag="pp")
                        nc.tensor.matmul(out=pp[:], lhsT=xTs[:, 0:P],
                                         rhs=w1g_sb[:, 0, :], start=True, stop=False)
                        nc.tensor.matmul(out=pp[:], lhsT=xTs[:, P:D],
                                         rhs=w1g_sb[:, 1, :], start=False, stop=True)
                        nc.vector.tensor_scalar(
                            out=a_big[:, b, :], in0=pp[:], scalar1=rc[:, b:b + 1],
                            scalar2=None, op0=ALU.mult)
                        if cfg["has_bw1"]:
                            nc.vector.tensor_tensor(
                                out=a_big[:, b, :], in0=a_big[:, b, :],
                                in1=osb["bw1rep"][:], op=ALU.add)
                    nc.sync.dma_start(
                        out=adram[bt * 1024:(bt + 1) * 1024, :], in_=a_big[:])

                if phase == 'spill':
                    for z in range(12):
                        nc.sync.dma_start(
                            out=out_d[z * 1024:(z + 1) * 1024, :], in_=zt[:])
                    return
                # ---------- gather-accumulate over unique slots ----------
                acc = accpool.tile([P, NB, H], F32, tag="acc")
                nc.gpsimd.dma_gather(
                    acc[:], adram[:], g0_sb[:], U_pad, U_pad, H,
                    single_packet=SINGLE_PACKET)
                if phase == 'gather0':
                    nc.sync.dma_start(out=out_d[0:NB * 64, :], in_=acc[:])
                    for z in range(4):
                        nc.sync.dma_start(
                            out=out_d[N_NODE + z * 1024:N_NODE + (z + 1) * 1024, :],
                            in_=zt[:])
                    return
                for k in range(1, K_g):
                    nbk = mks[k - 1] // P
                    stg = stgpool.tile([P, nbk, H], F32, tag="stg")
                    nc.gpsimd.dma_gather(
                        stg[:], adram[:], gk_sb[k - 1][:], mks[k - 1],
                        mks[k - 1], H, single_packet=SINGLE_PACKET)
                    nc.vector.tensor_tensor(
                        out=acc[:, 0:nbk, :], in0=acc[:, 0:nbk, :], in1=stg[:],
                        op=ALU.add)
                if cfg["has_b1"]:
                    for blk in range(NB):
                        nc.vector.tensor_tensor(
                            out=acc[:, blk, :], in0=acc[:, blk, :],
                            in1=osb["b1rep"][:], op=ALU.add)
                for blk0 in range(0, NB, 8):
                    blk1 = min(blk0 + 8, NB)
                    nc.scalar.activation(
                        acc[:, blk0:blk1, :], acc[:, blk0:blk1, :], AF.Gelu)

                if phase == 'gather':
                    nc.sync.dma_start(out=out_d[0:NB * 64, :], in_=acc[:])
                    for z in range(4):
                        nc.sync.dma_start(
                            out=out_d[N_NODE + z * 1024:N_NODE + (z + 1) * 1024, :],
                            in_=zt[:])
                    return
                # ---------- chunk-sum matmul + w2 ----------
                cps = ps_cs.tile([P, L], F32, space="PSUM", tag="cps")
                for blk in range(NB):
                    nc.tensor.matmul(out=cps[:], lhsT=acc[:, blk, :],
                                     rhs=mt_sb[:, blk, :],
                                     start=(blk == 0), stop=(blk == NB - 1))
                compT = dpool.tile([P, L], F32, tag="compT")
                nc.vector.tensor_copy(out=compT[:], in_=cps[:])
                c2ps = ps_sm.tile([C, L], F32, space="PSUM", tag="sm")
                nc.tensor.matmul(out=c2ps[:], lhsT=w2_sb[:], rhs=compT[:],
                                 start=True, stop=False)
                nc.tensor.matmul(out=c2ps[:], lhsT=b2r_sb[:], rhs=ones16[:],
                                 start=False, stop=True)
                c2 = dpool.tile([C, L], F32, tag="c2")
                nc.vector.tensor_copy(out=c2[:], in_=c2ps[:])

                # ---------- LNf over the flattened [16*64] ----------
                junk = dpool.tile([C, L], F32, tag="junk")
                rs = dpool.tile([C, 1], F32, tag="rs")
                sqs = dpool.tile([C, 1], F32, tag="sqs")
                nc.scalar.activation(junk[:], c2[:], AF.Identity, accum_out=rs[:])
                nc.scalar.activation(junk[:], c2[:], AF.Square, accum_out=sqs[:])
                t1ps = ps_sm.tile([1, 1], F32, space="PSUM", tag="sm")
                t2ps = ps_sm.tile([1, 1], F32, space="PSUM", tag="sm")
                nc.tensor.matmul(out=t1ps[:], lhsT=rs[:], rhs=onescol[:],
                                 start=True, stop=True)
                nc.tensor.matmul(out=t2ps[:], lhsT=sqs[:], rhs=onescol[:],
                                 start=True, stop=True)
                mean = dpool.tile([1, 1], F32, tag="mean")
                msq = dpool.tile([1, 1], F32, tag="msq")
                nc.vector.tensor_scalar_mul(mean[:], t1ps[:], 1.0 / (L * C))
                nc.vector.tensor_scalar_mul(msq[:], t2ps[:], 1.0 / (L * C))
                var = dpool.tile([1, 1], F32, tag="var")
                nc.vector.tensor_tensor(out=var[:], in0=mean[:], in1=mean[:],
                                        op=ALU.mult)
                nc.vector.tensor_tensor(out=var[:], in0=msq[:], in1=var[:],
                                        op=ALU.subtract)
                rstd = dpool.tile([1, 1], F32, tag="rstd")
                nc.vector.tensor_scalar_add(rstd[:], var[:], EPS)
                nc.scalar.sqrt(rstd[:], rstd[:])
                nc.vector.reciprocal(rstd[:], rstd[:])
                nmr = dpool.tile([1, 1], F32, tag="nmr")
                nc.vector.tensor_scalar(out=nmr[:], in0=mean[:], scalar1=rstd[:],
                                        scalar2=-1.0, op0=ALU.mult, op1=ALU.mult)
                bc_r = dpool.tile([C, 1], F32, tag="bc_r")
                bc_n = dpool.tile([C, 1], F32, tag="bc_n")
                nc.gpsimd.partition_broadcast(bc_r[:], rstd[:])
                nc.gpsimd.partition_broadcast(bc_n[:], nmr[:])
                c2n = dpool.tile([C, L], F32, tag="c2n")
                nc.scalar.activation(c2n[:], c2[:], AF.Identity,
                                     bias=bc_n[:], scale=bc_r[:])
                if not cfg["lnf_triv"]:
                    nc.vector.tensor_tensor(out=c2n[:], in0=c2n[:],
                                            in1=osb["lnfg"][:], op=ALU.mult)
                    nc.vector.tensor_tensor(out=c2n[:], in0=c2n[:],
                                            in1=osb["lnfb"][:], op=ALU.add)

                # ---------- LNd per row + decode MLP (tiny) ----------
                cfps = ps_sm.tile([L, C], F32, space="PSUM", tag="sm")
                nc.tensor.transpose(out=cfps[:], in_=c2n[:], identity=ident[0:C, 0:C])
                cf = dpool.tile([L, C], F32, tag="cf")
                nc.vector.tensor_copy(out=cf[:], in_=cfps[:])
                st2 = dpool.tile([L, 6], F32, tag="st2")
                mv2 = dpool.tile([L, 2], F32, tag="mv2")
                nc.vector.bn_stats(st2[:], cf[:])
                nc.vector.bn_aggr(mv2[:], st2[:])
                rc2 = dpool.tile([L, 1], F32, tag="rc2")
                nm2 = dpool.tile([L, 1], F32, tag="nm2")
                nc.vector.tensor_scalar_add(rc2[:], mv2[:, 1:2], EPS)
                nc.scalar.sqrt(rc2[:], rc2[:])
                nc.vector.reciprocal(rc2[:], rc2[:])
                nc.vector.tensor_scalar(out=nm2[:], in0=mv2[:, 0:1], scalar1=rc2[:],
                                        scalar2=-1.0, op0=ALU.mult, op1=ALU.mult)
                t2n = dpool.tile([L, C], F32, tag="t2n")
                nc.scalar.activation(t2n[:], cf[:], AF.Identity,
                                     bias=nm2[:], scale=rc2[:])
                if not cfg["lnd_triv"]:
                    nc.vector.tensor_tensor(out=t2n[:], in0=t2n[:],
                                            in1=osb["lndg"][:], op=ALU.mult)
                    nc.vector.tensor_tensor(out=t2n[:], in0=t2n[:],
                                            in1=osb["lndb"][:], op=ALU.add)
                ttps = ps_sm.tile([C, L], F32, space="PSUM", tag="sm")
                nc.tensor.transpose(out=ttps[:], in_=t2n[:], identity=ident[0:L, 0:L])
                t2nT = dpool.tile([C, L], F32, tag="t2nT")
                nc.vector.tensor_copy(out=t2nT[:], in_=ttps[:])

                d1ps = ps_mm.tile([P, L], F32, space="PSUM", tag="pp")
                nc.tensor.matmul(out=d1ps[:], lhsT=dw1_sb[:], rhs=t2nT[:],
                                 start=True, stop=False)
                nc.tensor.matmul(out=d1ps[:], lhsT=db1r_sb[:], rhs=ones16[:],
                                 start=False, stop=True)
                d1T = dpool.tile([P, L], F32, tag="d1T")
                nc.scalar.activation(d1T[:], d1ps[:], AF.Gelu)
                decps = ps_out.tile([L, D], F32, space="PSUM", tag="ops")
                nc.tensor.matmul(out=decps[:], lhsT=d1T[:], rhs=dw2_sb[:],
                                 start=True, stop=False)
                nc.tensor.matmul(out=decps[:], lhsT=ones16[:], rhs=db2r_sb[:],
                                 start=False, stop=True)
                dec = dpool.tile([L, D], F32, tag="dec")
                nc.vector.tensor_copy(out=dec[:], in_=decps[:])
                dhl = dpool.tile([4 * L, D], BF16, tag="dhl")
                nc.vector.memset(dhl[:], 0.0)
                nc.vector.tensor_copy(out=dhl[0:L, :], in_=dec[:])
                dhi32 = dpool.tile([L, D], F32, tag="dhi32")
                nc.vector.tensor_copy(out=dhi32[:], in_=dhl[0:L, :])
                dlo = dpool.tile([L, D], F32, tag="dlo")
                nc.vector.tensor_tensor(out=dlo[:], in0=dec[:], in1=dhi32[:],
                                        op=ALU.subtract)
                nc.vector.tensor_copy(out=dhl[2 * L:3 * L, :], in_=dlo[:])

                if phase == 'dec':
                    nc.sync.dma_start(out=out_d[0:64, :], in_=dhl[:].bitcast(F32))
                    for z in range(4):
                        nc.sync.dma_start(
                            out=out_d[N_NODE + z * 1024:N_NODE + (z + 1) * 1024, :],
                            in_=zt[:])
                    return
                # ---------- output gather (one-hot matmul) + zeros ----------
                for bt in range(NBT):
                    ob = opool.tile([P, B, D], F32, tag="ob")
                    for b in range(B):
                        col = (bt * B + b) * P
                        ops_ = ps_out.tile([P, D], F32, space="PSUM", tag="ops")
                        nc.tensor.matmul(out=ops_[:], lhsT=oh_sb[:, col:col + P],
                                         rhs=dhl[:], start=True, stop=True)
                        if b % 2 == 0:
                            nc.vector.tensor_copy(out=ob[:, b, :], in_=ops_[:])
                        else:
                            nc.scalar.copy(out=ob[:, b, :], in_=ops_[:])
                    nc.sync.dma_start(
                        out=out_d[bt * 1024:(bt + 1) * 1024, :], in_=ob[:])
                for z in range(4):
                    nc.sync.dma_start(
                        out=out_d[N_NODE + z * 1024:N_NODE + (z + 1) * 1024, :],
                        in_=zt[:])

            if loop_k > 0:
                with tc.For_i(0, loop_k, 1,
                              hint_engines=(mybir.EngineType.PE,
                                            mybir.EngineType.DVE,
                                            mybir.EngineType.Activation,
                                            mybir.EngineType.Pool,
                                            mybir.EngineType.SP)):
                    body()
            else:
                body()

    nc.compile()
    return nc


def kernel(**inputs) -> np.ndarray:
    cfg, in_maps = host_prep(**inputs)
    nc = build(cfg)
    res = bass_utils.run_bass_kernel_spmd(nc, in_maps, core_ids=list(range(T)))
    out = np.stack([res.results[c]["out"] for c in range(T)], axis=0)
    return out.astype(np.float32)
